# revision 19
# baseline (speedup 1.0000x reference)
"""Trainium2 Bass kernel: dense transformer block (attention + per-batch bmm + FF).

Sharding: 8 cores = (batch b = c//2) x (query-half nh = c%2).
Each core computes attention for all 16 heads over its 1024 query rows
(keys/values over full S=2048, recomputed per batch-pair), then the
per-batch feature-reduction bmm and the feed-forward for its rows.

All heavy matmuls run as float32r (fp22 mantissa, full PE rate at free>=256).
Scores are computed twice. Pass 1 feeds the row-max path: pair-wise max of
two PSUM score tiles per DVE tensor_tensor (one TT consumes 1024 elements),
then a bf16 SBUF combine tree + partition_all_reduce + u-row copy all on the
GPSIMD/Pool engine.  Pass 2 re-computes scores with a 65th contraction row
(k_aug row = -1, q_aug row = +rowmax) so the shift is baked into PSUM and
ScalarE does shift+exp+evacuate in one activation per m-tile.  V carries a
65th ones-column so the softmax denominator falls out of the attn@v matmul;
z is normalized via a K=1 broadcast matmul of the reciprocal + one
tensor_tensor multiply.  Blocks are software-pipelined: pass-1 matmuls of
block B interleave with pass-2/attn@v matmuls of block B-1 at m-tile
granularity so the PE never waits on the max path.
"""

import sys

sys.path.insert(0, "/opt/trn_rl_repo")

import numpy as np

B, S, E, H, HF = 4, 2048, 1024, 16, 64
NH = 1024          # query rows per core
SCALE = 1.0 / np.sqrt(HF)
NMT = S // 128     # key m-tiles (16)

_CACHE = {}
LABELS = {}
_CST = np.concatenate([np.ones((1, S), np.float32), -np.ones((1, S), np.float32)])


def _np_reference(x, attention_mask, Wq, Wk, Wv, Wr, Wff, bff):
    """Fallback (used only if the mask is not all-ones)."""
    x64 = x.astype(np.float64)
    q = np.einsum("bse,hef->bhsf", x64, Wq.astype(np.float64)).reshape(B * H, S, HF)
    k = np.einsum("bse,hef->bhsf", x64, Wk.astype(np.float64)).reshape(B * H, S, HF)
    v = np.einsum("bse,hef->bhsf", x64, Wv.astype(np.float64)).reshape(B * H, S, HF)
    s = np.matmul(q, k.transpose(0, 2, 1))
    s = np.where(attention_mask[0] == 0, -1e9, s)
    s = s * SCALE
    s = s - s.max(axis=-1, keepdims=True)
    p = np.exp(s)
    p /= p.sum(axis=-1, keepdims=True)
    z = np.matmul(p, v).reshape(B, H, S, HF).transpose(0, 2, 1, 3).reshape(B, S, E)
    z = np.matmul(z, Wr.astype(np.float64))
    o = np.maximum(z @ Wff.astype(np.float64).T + bff.astype(np.float64), 0.0)
    return o.astype(np.float32)


def _build():
    import concourse.bacc as bacc
    import concourse.bass as bass
    import concourse.mybir as mybir
    import concourse.tile as tile
    import bass_rust

    F32 = mybir.dt.float32
    BF16 = mybir.dt.bfloat16
    F32R = mybir.dt.float32r
    MULT = mybir.AluOpType.mult
    MAXOP = mybir.AluOpType.max
    EXP = mybir.ActivationFunctionType.Exp
    RELU = mybir.ActivationFunctionType.Relu
    RMAX = bass_rust.ReduceOp.max
    PSUM = bass.MemorySpace.PSUM

    def r(ap):
        return ap.bitcast(F32R)

    def L(label, inst):
        try:
            LABELS[inst.ins.name] = label
        except Exception:
            pass
        return inst

    nc = bacc.Bacc("TRN2", target_bir_lowering=False, debug=False)
    xt_d = nc.dram_tensor("xt", [E, S], F32, kind="ExternalInput")      # x[b].T, cols rolled so my half is first
    wq_d = nc.dram_tensor("wq", [E, E], F32, kind="ExternalInput")      # [e, h*HF+f]
    wk_d = nc.dram_tensor("wk", [E, E], F32, kind="ExternalInput")
    wv_d = nc.dram_tensor("wv", [E, E], F32, kind="ExternalInput")
    wr_d = nc.dram_tensor("wr", [E, E], F32, kind="ExternalInput")      # Wr[b]
    wfft_d = nc.dram_tensor("wfft", [E, E], F32, kind="ExternalInput")  # Wff.T
    bff_d = nc.dram_tensor("bff", [E, 1], F32, kind="ExternalInput")
    cst_d = nc.dram_tensor("cst", [2, S], F32, kind="ExternalInput")    # rows: 1.0, -1.0
    out_d = nc.dram_tensor("o", [E, NH], F32, kind="ExternalOutput")    # [j, n]

    with tile.TileContext(nc) as tc:
        with tc.tile_pool(name="glob", bufs=1) as glob:
            zTn = glob.tile([128, 8, NH], BF16)     # normalized z^T: [f-in-pair, echunk, n]
            bfft = glob.tile([128, 8], F32)
            nc.sync.dma_start(out=bfft, in_=bff_d.ap().rearrange("(t p) o -> p (t o)", p=128))
            ones65 = glob.tile([1, 65], F32)
            nc.sync.dma_start(out=r(ones65), in_=r(cst_d.ap())[0:1, 0:65])

            # ---------------- phase 1: projections + attention ----------------
            with tc.tile_pool(name="p1x", bufs=1) as p1x, \
                 tc.tile_pool(name="wpool", bufs=2) as wpool, \
                 tc.tile_pool(name="wvpool", bufs=1) as wvpool, \
                 tc.tile_pool(name="qkpool", bufs=2) as qkpool, \
                 tc.tile_pool(name="vpool", bufs=2) as vpool, \
                 tc.tile_pool(name="scr", bufs=1) as scr, \
                 tc.tile_pool(name="epool", bufs=3) as epool, \
                 tc.tile_pool(name="spool", bufs=1) as spool, \
                 tc.tile_pool(name="evac", bufs=2) as evac, \
                 tc.tile_pool(name="ps_a", bufs=2, space=PSUM) as ps_a, \
                 tc.tile_pool(name="ps_s2", bufs=2, space=PSUM) as ps_s2, \
                 tc.tile_pool(name="ps_z", bufs=2, space=PSUM) as ps_z:

                xt = p1x.tile([128, 8, S], F32)    # 64KB/part
                dma_engines = None  # set after emit_wdma defined

                # per-hp state built by emit_proj
                state = {}

                def emit_wdma(hp):
                    """Prefetch wq/wk (and wv for even hp) for head-pair hp."""
                    wq_sb = wpool.tile([128, 8, 128], F32, tag="wq")
                    nc.sync.dma_start(out=r(wq_sb), in_=r(wq_d.ap()).rearrange("(i p) c -> p i c", p=128)[:, :, hp * 128:(hp + 1) * 128])
                    wk_sb = wpool.tile([128, 8, 128], F32, tag="wk")
                    nc.sync.dma_start(out=r(wk_sb), in_=r(wk_d.ap()).rearrange("(i p) c -> p i c", p=128)[:, :, hp * 128:(hp + 1) * 128])
                    wv_sb = None
                    if hp % 2 == 0:
                        g = hp // 2
                        wv_sb = wvpool.tile([128, 8, 256], F32, tag="wv")
                        nc.sync.dma_start(out=r(wv_sb), in_=r(wv_d.ap()).rearrange("(i p) c -> p i c", p=128)[:, :, g * 256:(g + 1) * 256])
                    state[hp] = {"wq": wq_sb, "wk": wk_sb, "wv": wv_sb}

                def emit_proj(hp):
                    """Projections for head pair hp: q into qaug, k into kaug, v (per group)."""
                    st = state[hp]
                    qaug = [qkpool.tile([65, NH], F32, tag=f"qaug{i}", name=f"qaug{i}") for i in range(2)]
                    kaug = [qkpool.tile([65, S], F32, tag=f"kaug{i}", name=f"kaug{i}") for i in range(2)]
                    nc.sync.dma_start(out=r(kaug[0][64:65, :]), in_=r(cst_d.ap())[1:2, :])
                    nc.sync.dma_start(out=r(kaug[1][64:65, :]), in_=r(cst_d.ap())[1:2, :])
                    psq = ps_a.tile([128, 2, 512], F32, tag="psa", name="psq")
                    for nb in range(2):
                        sl = slice(nb * 512, (nb + 1) * 512)
                        for e in range(8):
                            L(f"projq hp{hp} nb{nb} e{e}", nc.tensor.matmul(psq[:, nb, :], r(st["wq"][:, e, :]), r(xt[:, e, nb * 512:(nb + 1) * 512]),
                                             start=(e == 0), stop=(e == 7)))
                        qx = evac.tile([128, 512], F32, tag="qx", name="qx")
                        nc.scalar.copy(out=r(qx), in_=psq[:, nb, :])
                        nc.sync.dma_start(out=r(qaug[0][0:64, sl]), in_=r(qx[0:64, :]))
                        nc.sync.dma_start(out=r(qaug[1][0:64, sl]), in_=r(qx[64:128, :]))
                    for mbp in range(2):
                        psk = ps_a.tile([128, 2, 512], F32, tag="psa", name="psk")
                        for mh in range(2):
                            mb = 2 * mbp + mh
                            sl = slice(mb * 512, (mb + 1) * 512)
                            for e in range(8):
                                L(f"projk hp{hp} mb{mb} e{e}", nc.tensor.matmul(psk[:, mh, :], r(st["wk"][:, e, :]), r(xt[:, e, mb * 512:(mb + 1) * 512]),
                                                 start=(e == 0), stop=(e == 7)))
                            kx = evac.tile([128, 512], F32, tag="qx", name="kx")
                            nc.scalar.copy(out=r(kx), in_=psk[:, mh, :])
                            nc.sync.dma_start(out=r(kaug[0][0:64, sl]), in_=r(kx[0:64, :]))
                            nc.sync.dma_start(out=r(kaug[1][0:64, sl]), in_=r(kx[64:128, :]))
                    st["qaug"], st["kaug"] = qaug, kaug
                    if hp % 2 == 0:
                        v4 = vpool.tile([128, NMT, 4, 65], BF16, tag="v4")
                        nc.vector.memset(v4[:, :, :, 64:65], 1.0)
                        for mtp in range(NMT // 2):
                            psv = ps_a.tile([128, 2, 512], F32, tag="psa", name="psv")
                            for mh in range(2):
                                mt = 2 * mtp + mh
                                for e in range(8):
                                    L(f"projv hp{hp} mt{mt} e{e}", nc.tensor.matmul(psv[:, mh, 0:256], r(xt[:, e, mt * 128:(mt + 1) * 128]), r(st["wv"][:, e, :]),
                                                     start=(e == 0), stop=(e == 7)))
                                nc.scalar.copy(out=v4[:, mt, :, 0:64], in_=psv[:, mh, 0:256].rearrange("p (s f) -> p s f", s=4))
                        state["v4"] = v4
                    st["v4"] = state["v4"]

                def emit_xt():
                    engs = [nc.scalar, nc.gpsimd, nc.sync]
                    for e in range(8):
                        engs[e % 3].dma_start(out=r(xt[:, e, :]),
                                              in_=r(xt_d.ap()).rearrange("(i p) m -> p i m", p=128)[:, e, :])

                class Blk:
                    pass

                def new_block(hp, i, nb):
                    b = Blk()
                    b.hp, b.i, b.nb = hp, i, nb
                    b.h = 2 * hp + i
                    b.nsl = slice(nb * 512, (nb + 1) * 512)
                    st = state[hp]
                    b.qaug, b.kaug, b.v4 = st["qaug"][i], st["kaug"][i], st["v4"]
                    b.g, b.s4 = b.h // 4, b.h % 4
                    b.acc = spool.tile([128, 1024], BF16, tag="acc", name="acc", bufs=2)
                    b.pair = None
                    b.expt = [None] * NMT
                    b.zps = None
                    return b

                def emit_p1(b, t):
                    if t % 2 == 0:
                        b.pair = ps_a.tile([128, 2, 512], F32, tag="psa", name="p1ps")
                    L(f"p1 h{b.h} nb{b.nb} t{t}", nc.tensor.matmul(b.pair[:, t % 2, :], r(b.kaug[0:64, t * 128:(t + 1) * 128]), r(b.qaug[0:64, b.nsl]),
                                     start=True, stop=True))
                    if t % 2 == 1:
                        wide = b.pair.rearrange("p a b -> p (a b)")
                        if t == 1:
                            nc.vector.tensor_copy(out=b.acc, in_=wide)
                        else:
                            nc.vector.tensor_tensor(out=b.acc, in0=wide, in1=b.acc, op=MAXOP)

                def emit_tree(b):
                    """Fold acc halves (DVE bf16 2x), partition all-reduce + u-row copy on Pool."""
                    L4 = scr.tile([128, 512], BF16, tag="L4", name="L4")
                    nc.vector.tensor_tensor(out=L4, in0=b.acc[:, 0:512], in1=b.acc[:, 512:1024], op=MAXOP)
                    mall = scr.tile([128, 512], BF16, tag="mall", name="mall")
                    nc.gpsimd.partition_all_reduce(mall, L4, channels=128, reduce_op=RMAX)
                    nc.gpsimd.tensor_copy(out=r(b.qaug[64:65, b.nsl]), in_=mall[64:65, :])

                def emit_p2(b, t):
                    ps2 = ps_s2.tile([128, 512], F32, tag="ps2", name="ps2")
                    L(f"p2 h{b.h} nb{b.nb} t{t}", nc.tensor.matmul(ps2, r(b.kaug[0:65, t * 128:(t + 1) * 128]), r(b.qaug[0:65, b.nsl]),
                                     start=True, stop=True))
                    ex = epool.tile([128, 512], BF16, tag="expt", name="expt")
                    nc.scalar.activation(out=ex, in_=ps2, func=EXP, scale=float(SCALE))
                    b.expt[t] = ex

                def emit_av(b, t):
                    if t == 0:
                        b.zps = ps_z.tile([65, 512], F32, tag="zr", name="zps")
                    L(f"av h{b.h} nb{b.nb} t{t}", nc.tensor.matmul(b.zps, b.v4[:, t, b.s4, :], b.expt[t],
                                     start=(t == 0), stop=(t == NMT - 1)))
                    b.expt[t] = None

                def emit_recip(b):
                    """Stage 1 of normalization: reciprocal of the denominator row."""
                    b.rec = spool.tile([1, 512], F32, tag="rec", name="rec")
                    with nc.allow_low_precision(reason="softmax denominators are O(1)..O(1e3)"):
                        nc.vector.reciprocal(out=r(b.rec), in_=b.zps[64:65, :])

                def emit_bcast(b):
                    """Stage 2: broadcast reciprocal row to 64 partitions on Pool."""
                    b.recb = evac.tile([128, 512], F32, tag="qx", name="recb")
                    nc.gpsimd.partition_broadcast(b.recb[0:64, :], b.rec, channels=64)

                def emit_normfin(b):
                    """Stage 3: z * (1/denom) -> zTn (bf16)."""
                    ec = b.h // 2
                    if b.h % 2 == 0:
                        nc.vector.tensor_tensor(out=zTn[0:64, ec, b.nsl], in0=b.zps[0:64, :], in1=b.recb[0:64, :], op=MULT)
                    else:
                        ztmp = spool.tile([64, 512], BF16, tag="ztmp", name="ztmp")
                        nc.vector.tensor_tensor(out=ztmp, in0=b.zps[0:64, :], in1=b.recb[0:64, :], op=MULT)
                        nc.sync.dma_start(out=zTn[64:128, ec, b.nsl], in_=ztmp)

                # ---- software-pipelined main loop (p2/av lag two blocks behind p1) ----
                blocks = [(hp, i, nb) for hp in range(8) for i in range(2) for nb in range(2)]
                emit_wdma(0)
                emit_xt()
                ring = []          # processed blocks, ring[k] = block k
                for bi, (hp, i, nb) in enumerate(blocks):
                    if i == 0 and nb == 0:
                        if hp + 1 < 8:
                            emit_wdma(hp + 1)
                        emit_proj(hp)
                    b = new_block(hp, i, nb)
                    pv = ring[bi - 2] if bi >= 2 else None
                    for t in range(NMT):
                        emit_p1(b, t)
                        if t == 3 and bi >= 3:
                            emit_normfin(ring[bi - 3])
                        if pv is not None:
                            emit_p2(pv, t)
                            if t >= 2:
                                emit_av(pv, t - 2)
                    if pv is not None:
                        emit_av(pv, NMT - 2)
                        emit_av(pv, NMT - 1)
                        emit_recip(pv)
                    emit_tree(b)
                    if pv is not None:
                        emit_bcast(pv)
                    ring.append(b)
                # drain: p2/av for the last two blocks
                n = len(blocks)
                for k in (n - 2, n - 1):
                    pv = ring[k]
                    for t in range(NMT):
                        if t == 3:
                            emit_normfin(ring[k - 1])
                        emit_p2(pv, t)
                        if t >= 2:
                            emit_av(pv, t - 2)
                    emit_av(pv, NMT - 2)
                    emit_av(pv, NMT - 1)
                    emit_recip(pv)
                    emit_bcast(pv)
                emit_normfin(ring[n - 1])

            # ---------------- phase 2: y^T = Wr^T-style bmm, then FF ----------------
            with tc.tile_pool(name="p2w", bufs=1) as p2w, \
                 tc.tile_pool(name="p2y", bufs=1) as p2y, \
                 tc.tile_pool(name="p2o", bufs=2) as p2o, \
                 tc.tile_pool(name="ps_y", bufs=2, space=PSUM) as ps_y:
                wr_sb = p2w.tile([128, 8, E], F32, tag="wr")
                for dt in range(8):
                    eng = nc.sync if dt % 2 == 0 else nc.scalar
                    eng.dma_start(out=r(wr_sb[:, :, dt * 128:(dt + 1) * 128]),
                                  in_=r(wr_d.ap()).rearrange("(i p) c -> p i c", p=128)[:, :, dt * 128:(dt + 1) * 128])
                wff_sb = p2w.tile([128, 8, E], F32, tag="wff")
                for jt in range(8):
                    eng = nc.sync if jt % 2 == 0 else nc.scalar
                    eng.dma_start(out=r(wff_sb[:, :, jt * 128:(jt + 1) * 128]),
                                  in_=r(wfft_d.ap()).rearrange("(i p) c -> p i c", p=128)[:, :, jt * 128:(jt + 1) * 128])
                yT = p2y.tile([128, 8, NH], F32)
                for dt in range(8):
                    psy = ps_y.tile([128, NH], F32, tag="psy")
                    for ec in range(8):
                        for half in range(2):
                            L(f"wr dt{dt} ec{ec} h{half}", nc.tensor.matmul(psy[:, half * 512:(half + 1) * 512],
                                             r(wr_sb[:, ec, dt * 128:(dt + 1) * 128]),
                                             zTn[:, ec, half * 512:(half + 1) * 512],
                                             start=(ec == 0), stop=(ec == 7)))
                    nc.vector.tensor_copy(out=r(yT[:, dt, :]), in_=psy)
                for jt in range(8):
                    pso = ps_y.tile([128, NH], F32, tag="psy")
                    for dc in range(8):
                        for half in range(2):
                            L(f"ff jt{jt} dc{dc} h{half}", nc.tensor.matmul(pso[:, half * 512:(half + 1) * 512],
                                             r(wff_sb[:, dc, jt * 128:(jt + 1) * 128]),
                                             r(yT[:, dc, half * 512:(half + 1) * 512]),
                                             start=(dc == 0), stop=(dc == 7)))
                    ot = p2o.tile([128, NH], F32, tag="ot")
                    nc.scalar.activation(out=ot, in_=pso, func=RELU, bias=bfft[:, jt:jt + 1], scale=1.0)
                    nc.sync.dma_start(out=out_d.ap()[jt * 128:(jt + 1) * 128, :], in_=ot)

    nc.compile()
    return nc


def _get_module():
    if "nc" not in _CACHE:
        _CACHE["nc"] = _build()
    return _CACHE["nc"]


def kernel(x, attention_mask, Wq, Wk, Wv, Wr, Wff, bff):
    from concourse import bass_utils

    x = np.asarray(x, dtype=np.float32)
    attention_mask = np.asarray(attention_mask)
    Wq = np.asarray(Wq, dtype=np.float32)
    Wk = np.asarray(Wk, dtype=np.float32)
    Wv = np.asarray(Wv, dtype=np.float32)
    Wr = np.asarray(Wr, dtype=np.float32)
    Wff = np.asarray(Wff, dtype=np.float32)
    bff = np.asarray(bff, dtype=np.float32)

    if not np.all(attention_mask == 1):
        return _np_reference(x, attention_mask, Wq, Wk, Wv, Wr, Wff, bff)

    nc = _get_module()
    wq2 = np.ascontiguousarray(Wq.transpose(1, 0, 2).reshape(E, E))
    wk2 = np.ascontiguousarray(Wk.transpose(1, 0, 2).reshape(E, E))
    wv2 = np.ascontiguousarray(Wv.transpose(1, 0, 2).reshape(E, E))
    wfft = np.ascontiguousarray(Wff.T)
    bff2 = np.ascontiguousarray(bff.reshape(E, 1))

    in_maps = []
    for c in range(8):
        b, nh = c // 2, c % 2
        xt = x[b].T
        if nh:
            xt = np.concatenate([xt[:, NH:], xt[:, :NH]], axis=1)
        in_maps.append({
            "xt": np.ascontiguousarray(xt),
            "wq": wq2, "wk": wk2, "wv": wv2,
            "wr": np.ascontiguousarray(Wr[b]),
            "wfft": wfft, "bff": bff2,
            "cst": _CST,
        })

    res = bass_utils.run_bass_kernel_spmd(nc, in_maps, core_ids=list(range(8)), **_CACHE.get("run_kwargs", {}))
    _CACHE["last_result"] = res

    out = np.empty((B, S, E), dtype=np.float32)
    for c in range(8):
        b, nh = c // 2, c % 2
        out[b, nh * NH:(nh + 1) * NH, :] = res.results[c]["o"].T
    return out


# revision 20
# speedup vs baseline: 1.0064x; 1.0064x over previous
"""Trainium2 Bass kernel: dense transformer block (attention + per-batch bmm + FF).

Sharding: 8 cores = (batch b = c//2) x (query-half nh = c%2).
Each core computes attention for all 16 heads over its 1024 query rows
(keys/values over full S=2048, recomputed per batch-pair), then the
per-batch feature-reduction bmm and the feed-forward for its rows.

All heavy matmuls run as float32r (fp22 mantissa, full PE rate at free>=256).
Scores are computed twice. Pass 1 feeds the row-max path: pair-wise max of
two PSUM score tiles per DVE tensor_tensor (one TT consumes 1024 elements),
then a bf16 SBUF combine tree + partition_all_reduce + u-row copy all on the
GPSIMD/Pool engine.  Pass 2 re-computes scores with a 65th contraction row
(k_aug row = -1, q_aug row = +rowmax) so the shift is baked into PSUM and
ScalarE does shift+exp+evacuate in one activation per m-tile.  V carries a
65th ones-column so the softmax denominator falls out of the attn@v matmul;
z is normalized via a K=1 broadcast matmul of the reciprocal + one
tensor_tensor multiply.  Blocks are software-pipelined: pass-1 matmuls of
block B interleave with pass-2/attn@v matmuls of block B-1 at m-tile
granularity so the PE never waits on the max path.
"""

import sys

sys.path.insert(0, "/opt/trn_rl_repo")

import numpy as np

B, S, E, H, HF = 4, 2048, 1024, 16, 64
NH = 1024          # query rows per core
SCALE = 1.0 / np.sqrt(HF)
NMT = S // 128     # key m-tiles (16)

_CACHE = {}
LABELS = {}
_CST = np.concatenate([np.ones((1, S), np.float32), -np.ones((1, S), np.float32)])


def _np_reference(x, attention_mask, Wq, Wk, Wv, Wr, Wff, bff):
    """Fallback (used only if the mask is not all-ones)."""
    x64 = x.astype(np.float64)
    q = np.einsum("bse,hef->bhsf", x64, Wq.astype(np.float64)).reshape(B * H, S, HF)
    k = np.einsum("bse,hef->bhsf", x64, Wk.astype(np.float64)).reshape(B * H, S, HF)
    v = np.einsum("bse,hef->bhsf", x64, Wv.astype(np.float64)).reshape(B * H, S, HF)
    s = np.matmul(q, k.transpose(0, 2, 1))
    s = np.where(attention_mask[0] == 0, -1e9, s)
    s = s * SCALE
    s = s - s.max(axis=-1, keepdims=True)
    p = np.exp(s)
    p /= p.sum(axis=-1, keepdims=True)
    z = np.matmul(p, v).reshape(B, H, S, HF).transpose(0, 2, 1, 3).reshape(B, S, E)
    z = np.matmul(z, Wr.astype(np.float64))
    o = np.maximum(z @ Wff.astype(np.float64).T + bff.astype(np.float64), 0.0)
    return o.astype(np.float32)


def _build():
    import concourse.bacc as bacc
    import concourse.bass as bass
    import concourse.mybir as mybir
    import concourse.tile as tile
    import bass_rust

    F32 = mybir.dt.float32
    BF16 = mybir.dt.bfloat16
    F32R = mybir.dt.float32r
    MULT = mybir.AluOpType.mult
    MAXOP = mybir.AluOpType.max
    EXP = mybir.ActivationFunctionType.Exp
    RELU = mybir.ActivationFunctionType.Relu
    RMAX = bass_rust.ReduceOp.max
    PSUM = bass.MemorySpace.PSUM

    def r(ap):
        return ap.bitcast(F32R)

    def L(label, inst):
        try:
            LABELS[inst.ins.name] = label
        except Exception:
            pass
        return inst

    nc = bacc.Bacc("TRN2", target_bir_lowering=False, debug=False)
    xt_d = nc.dram_tensor("xt", [E, S], F32, kind="ExternalInput")      # x[b].T, cols rolled so my half is first
    wq_d = nc.dram_tensor("wq", [E, E], F32, kind="ExternalInput")      # [e, h*HF+f]
    wk_d = nc.dram_tensor("wk", [E, E], F32, kind="ExternalInput")
    wv_d = nc.dram_tensor("wv", [E, E], F32, kind="ExternalInput")
    wr_d = nc.dram_tensor("wr", [E, E], F32, kind="ExternalInput")      # Wr[b]
    wfft_d = nc.dram_tensor("wfft", [E, E], F32, kind="ExternalInput")  # Wff.T
    bff_d = nc.dram_tensor("bff", [E, 1], F32, kind="ExternalInput")
    cst_d = nc.dram_tensor("cst", [2, S], F32, kind="ExternalInput")    # rows: 1.0, -1.0
    out_d = nc.dram_tensor("o", [E, NH], F32, kind="ExternalOutput")    # [j, n]

    with tile.TileContext(nc) as tc:
        with tc.tile_pool(name="glob", bufs=1) as glob:
            zTn = glob.tile([128, 8, NH], F32)     # normalized z^T: [f-in-pair, echunk, n]
            bfft = glob.tile([128, 8], F32)
            nc.sync.dma_start(out=bfft, in_=bff_d.ap().rearrange("(t p) o -> p (t o)", p=128))
            ones65 = glob.tile([1, 65], F32)
            nc.sync.dma_start(out=r(ones65), in_=r(cst_d.ap())[0:1, 0:65])

            # ---------------- phase 1: projections + attention ----------------
            with tc.tile_pool(name="p1x", bufs=1) as p1x, \
                 tc.tile_pool(name="wpool", bufs=2) as wpool, \
                 tc.tile_pool(name="wvpool", bufs=1) as wvpool, \
                 tc.tile_pool(name="qkpool", bufs=2) as qkpool, \
                 tc.tile_pool(name="vpool", bufs=2) as vpool, \
                 tc.tile_pool(name="scr", bufs=1) as scr, \
                 tc.tile_pool(name="epool", bufs=3) as epool, \
                 tc.tile_pool(name="spool", bufs=1) as spool, \
                 tc.tile_pool(name="evac", bufs=2) as evac, \
                 tc.tile_pool(name="ps_a", bufs=2, space=PSUM) as ps_a, \
                 tc.tile_pool(name="ps_s2", bufs=2, space=PSUM) as ps_s2, \
                 tc.tile_pool(name="ps_z", bufs=2, space=PSUM) as ps_z:

                xt = p1x.tile([128, 8, S], F32)    # 64KB/part
                dma_engines = None  # set after emit_wdma defined

                # per-hp state built by emit_proj
                state = {}

                def emit_wdma(hp):
                    """Prefetch wq/wk (and wv for even hp) for head-pair hp."""
                    wq_sb = wpool.tile([128, 8, 128], F32, tag="wq")
                    nc.sync.dma_start(out=r(wq_sb), in_=r(wq_d.ap()).rearrange("(i p) c -> p i c", p=128)[:, :, hp * 128:(hp + 1) * 128])
                    wk_sb = wpool.tile([128, 8, 128], F32, tag="wk")
                    nc.sync.dma_start(out=r(wk_sb), in_=r(wk_d.ap()).rearrange("(i p) c -> p i c", p=128)[:, :, hp * 128:(hp + 1) * 128])
                    wv_sb = None
                    if hp % 2 == 0:
                        g = hp // 2
                        wv_sb = wvpool.tile([128, 8, 256], F32, tag="wv")
                        nc.sync.dma_start(out=r(wv_sb), in_=r(wv_d.ap()).rearrange("(i p) c -> p i c", p=128)[:, :, g * 256:(g + 1) * 256])
                    state[hp] = {"wq": wq_sb, "wk": wk_sb, "wv": wv_sb}

                def emit_proj(hp):
                    """Projections for head pair hp: q into qaug, k into kaug, v (per group)."""
                    st = state[hp]
                    qaug = [qkpool.tile([65, NH], F32, tag=f"qaug{i}", name=f"qaug{i}") for i in range(2)]
                    kaug = [qkpool.tile([65, S], F32, tag=f"kaug{i}", name=f"kaug{i}") for i in range(2)]
                    nc.sync.dma_start(out=r(kaug[0][64:65, :]), in_=r(cst_d.ap())[1:2, :])
                    nc.sync.dma_start(out=r(kaug[1][64:65, :]), in_=r(cst_d.ap())[1:2, :])
                    psq = ps_a.tile([128, 2, 512], F32, tag="psa", name="psq")
                    for nb in range(2):
                        sl = slice(nb * 512, (nb + 1) * 512)
                        for e in range(8):
                            L(f"projq hp{hp} nb{nb} e{e}", nc.tensor.matmul(psq[:, nb, :], r(st["wq"][:, e, :]), r(xt[:, e, nb * 512:(nb + 1) * 512]),
                                             start=(e == 0), stop=(e == 7)))
                        qx = evac.tile([128, 512], F32, tag="qx", name="qx")
                        nc.scalar.copy(out=r(qx), in_=psq[:, nb, :])
                        nc.sync.dma_start(out=r(qaug[0][0:64, sl]), in_=r(qx[0:64, :]))
                        nc.sync.dma_start(out=r(qaug[1][0:64, sl]), in_=r(qx[64:128, :]))
                    for mbp in range(2):
                        psk = ps_a.tile([128, 2, 512], F32, tag="psa", name="psk")
                        for mh in range(2):
                            mb = 2 * mbp + mh
                            sl = slice(mb * 512, (mb + 1) * 512)
                            for e in range(8):
                                L(f"projk hp{hp} mb{mb} e{e}", nc.tensor.matmul(psk[:, mh, :], r(st["wk"][:, e, :]), r(xt[:, e, mb * 512:(mb + 1) * 512]),
                                                 start=(e == 0), stop=(e == 7)))
                            kx = evac.tile([128, 512], F32, tag="qx", name="kx")
                            nc.scalar.copy(out=r(kx), in_=psk[:, mh, :])
                            nc.sync.dma_start(out=r(kaug[0][0:64, sl]), in_=r(kx[0:64, :]))
                            nc.sync.dma_start(out=r(kaug[1][0:64, sl]), in_=r(kx[64:128, :]))
                    st["qaug"], st["kaug"] = qaug, kaug
                    if hp % 2 == 0:
                        v4 = vpool.tile([128, NMT, 4, 65], BF16, tag="v4")
                        nc.vector.memset(v4[:, :, :, 64:65], 1.0)
                        for mtp in range(NMT // 2):
                            psv = ps_a.tile([128, 2, 512], F32, tag="psa", name="psv")
                            for mh in range(2):
                                mt = 2 * mtp + mh
                                for e in range(8):
                                    L(f"projv hp{hp} mt{mt} e{e}", nc.tensor.matmul(psv[:, mh, 0:256], r(xt[:, e, mt * 128:(mt + 1) * 128]), r(st["wv"][:, e, :]),
                                                     start=(e == 0), stop=(e == 7)))
                                nc.scalar.copy(out=v4[:, mt, :, 0:64], in_=psv[:, mh, 0:256].rearrange("p (s f) -> p s f", s=4))
                        state["v4"] = v4
                    st["v4"] = state["v4"]

                def emit_xt():
                    engs = [nc.scalar, nc.gpsimd, nc.sync]
                    for e in range(8):
                        engs[e % 3].dma_start(out=r(xt[:, e, :]),
                                              in_=r(xt_d.ap()).rearrange("(i p) m -> p i m", p=128)[:, e, :])

                class Blk:
                    pass

                def new_block(hp, i, nb):
                    b = Blk()
                    b.hp, b.i, b.nb = hp, i, nb
                    b.h = 2 * hp + i
                    b.nsl = slice(nb * 512, (nb + 1) * 512)
                    st = state[hp]
                    b.qaug, b.kaug, b.v4 = st["qaug"][i], st["kaug"][i], st["v4"]
                    b.g, b.s4 = b.h // 4, b.h % 4
                    b.acc = spool.tile([128, 1024], BF16, tag="acc", name="acc", bufs=2)
                    b.pair = None
                    b.expt = [None] * NMT
                    b.zps = None
                    return b

                def emit_p1(b, t):
                    if t % 2 == 0:
                        b.pair = ps_a.tile([128, 2, 512], F32, tag="psa", name="p1ps")
                    L(f"p1 h{b.h} nb{b.nb} t{t}", nc.tensor.matmul(b.pair[:, t % 2, :], r(b.kaug[0:64, t * 128:(t + 1) * 128]), r(b.qaug[0:64, b.nsl]),
                                     start=True, stop=True))
                    if t % 2 == 1:
                        wide = b.pair.rearrange("p a b -> p (a b)")
                        if t == 1:
                            nc.vector.tensor_copy(out=b.acc, in_=wide)
                        else:
                            nc.vector.tensor_tensor(out=b.acc, in0=wide, in1=b.acc, op=MAXOP)

                def emit_tree(b):
                    """Fold acc halves (DVE bf16 2x), partition all-reduce + u-row copy on Pool."""
                    L4 = scr.tile([128, 512], BF16, tag="L4", name="L4")
                    nc.vector.tensor_tensor(out=L4, in0=b.acc[:, 0:512], in1=b.acc[:, 512:1024], op=MAXOP)
                    mall = scr.tile([128, 512], BF16, tag="mall", name="mall")
                    nc.gpsimd.partition_all_reduce(mall, L4, channels=128, reduce_op=RMAX)
                    nc.gpsimd.tensor_copy(out=r(b.qaug[64:65, b.nsl]), in_=mall[64:65, :])

                def emit_p2(b, t):
                    ps2 = ps_s2.tile([128, 512], F32, tag="ps2", name="ps2")
                    L(f"p2 h{b.h} nb{b.nb} t{t}", nc.tensor.matmul(ps2, r(b.kaug[0:65, t * 128:(t + 1) * 128]), r(b.qaug[0:65, b.nsl]),
                                     start=True, stop=True))
                    ex = epool.tile([128, 512], BF16, tag="expt", name="expt")
                    nc.scalar.activation(out=ex, in_=ps2, func=EXP, scale=float(SCALE))
                    b.expt[t] = ex

                def emit_av(b, t):
                    if t == 0:
                        b.zps = ps_z.tile([65, 512], F32, tag="zr", name="zps")
                    L(f"av h{b.h} nb{b.nb} t{t}", nc.tensor.matmul(b.zps, b.v4[:, t, b.s4, :], b.expt[t],
                                     start=(t == 0), stop=(t == NMT - 1)))
                    b.expt[t] = None

                def emit_recip(b):
                    """Stage 1 of normalization: reciprocal of the denominator row."""
                    b.rec = spool.tile([1, 512], F32, tag="rec", name="rec")
                    with nc.allow_low_precision(reason="softmax denominators are O(1)..O(1e3)"):
                        nc.vector.reciprocal(out=r(b.rec), in_=b.zps[64:65, :])

                def emit_bcast(b):
                    """Stage 2: broadcast reciprocal row to 64 partitions on Pool."""
                    b.recb = evac.tile([128, 512], F32, tag="qx", name="recb")
                    nc.gpsimd.partition_broadcast(b.recb[0:64, :], b.rec, channels=64)

                def emit_normfin(b):
                    """Stage 3: z * (1/denom) -> zTn (bf16)."""
                    ec = b.h // 2
                    if b.h % 2 == 0:
                        nc.vector.tensor_tensor(out=r(zTn[0:64, ec, b.nsl]), in0=b.zps[0:64, :], in1=b.recb[0:64, :], op=MULT)
                    else:
                        ztmp = spool.tile([64, 512], F32, tag="ztmp", name="ztmp")
                        nc.vector.tensor_tensor(out=r(ztmp), in0=b.zps[0:64, :], in1=b.recb[0:64, :], op=MULT)
                        nc.sync.dma_start(out=r(zTn[64:128, ec, b.nsl]), in_=r(ztmp))

                # ---- software-pipelined main loop (p2/av lag two blocks behind p1) ----
                blocks = [(hp, i, nb) for hp in range(8) for i in range(2) for nb in range(2)]
                emit_wdma(0)
                emit_xt()
                ring = []          # processed blocks, ring[k] = block k
                for bi, (hp, i, nb) in enumerate(blocks):
                    if i == 0 and nb == 0:
                        if hp + 1 < 8:
                            emit_wdma(hp + 1)
                        emit_proj(hp)
                    b = new_block(hp, i, nb)
                    pv = ring[bi - 2] if bi >= 2 else None
                    for t in range(NMT):
                        emit_p1(b, t)
                        if t == 3 and bi >= 3:
                            emit_normfin(ring[bi - 3])
                        if pv is not None:
                            emit_p2(pv, t)
                            if t >= 2:
                                emit_av(pv, t - 2)
                    if pv is not None:
                        emit_av(pv, NMT - 2)
                        emit_av(pv, NMT - 1)
                        emit_recip(pv)
                    emit_tree(b)
                    if pv is not None:
                        emit_bcast(pv)
                    ring.append(b)
                # drain: p2/av for the last two blocks
                n = len(blocks)
                for k in (n - 2, n - 1):
                    pv = ring[k]
                    for t in range(NMT):
                        if t == 3:
                            emit_normfin(ring[k - 1])
                        emit_p2(pv, t)
                        if t >= 2:
                            emit_av(pv, t - 2)
                    emit_av(pv, NMT - 2)
                    emit_av(pv, NMT - 1)
                    emit_recip(pv)
                    emit_bcast(pv)
                emit_normfin(ring[n - 1])

            # ---------------- phase 2: y^T = Wr^T-style bmm, then FF ----------------
            with tc.tile_pool(name="p2w", bufs=1) as p2w, \
                 tc.tile_pool(name="p2y", bufs=1) as p2y, \
                 tc.tile_pool(name="p2o", bufs=2) as p2o, \
                 tc.tile_pool(name="ps_y", bufs=2, space=PSUM) as ps_y:
                wr_sb = p2w.tile([128, 8, E], F32, tag="wr")
                for dt in range(8):
                    eng = nc.sync if dt % 2 == 0 else nc.scalar
                    eng.dma_start(out=r(wr_sb[:, :, dt * 128:(dt + 1) * 128]),
                                  in_=r(wr_d.ap()).rearrange("(i p) c -> p i c", p=128)[:, :, dt * 128:(dt + 1) * 128])
                wff_sb = p2w.tile([128, 8, E], F32, tag="wff")
                for jt in range(8):
                    eng = nc.sync if jt % 2 == 0 else nc.scalar
                    eng.dma_start(out=r(wff_sb[:, :, jt * 128:(jt + 1) * 128]),
                                  in_=r(wfft_d.ap()).rearrange("(i p) c -> p i c", p=128)[:, :, jt * 128:(jt + 1) * 128])
                yT = p2y.tile([128, 8, NH], F32)
                for dt in range(8):
                    psy = ps_y.tile([128, NH], F32, tag="psy")
                    for ec in range(8):
                        for half in range(2):
                            L(f"wr dt{dt} ec{ec} h{half}", nc.tensor.matmul(psy[:, half * 512:(half + 1) * 512],
                                             r(wr_sb[:, ec, dt * 128:(dt + 1) * 128]),
                                             r(zTn[:, ec, half * 512:(half + 1) * 512]),
                                             start=(ec == 0), stop=(ec == 7)))
                    nc.vector.tensor_copy(out=r(yT[:, dt, :]), in_=psy)
                for jt in range(8):
                    pso = ps_y.tile([128, NH], F32, tag="psy")
                    for dc in range(8):
                        for half in range(2):
                            L(f"ff jt{jt} dc{dc} h{half}", nc.tensor.matmul(pso[:, half * 512:(half + 1) * 512],
                                             r(wff_sb[:, dc, jt * 128:(jt + 1) * 128]),
                                             r(yT[:, dc, half * 512:(half + 1) * 512]),
                                             start=(dc == 0), stop=(dc == 7)))
                    ot = p2o.tile([128, NH], F32, tag="ot")
                    nc.scalar.activation(out=ot, in_=pso, func=RELU, bias=bfft[:, jt:jt + 1], scale=1.0)
                    nc.sync.dma_start(out=out_d.ap()[jt * 128:(jt + 1) * 128, :], in_=ot)

    nc.compile()
    return nc


def _get_module():
    if "nc" not in _CACHE:
        _CACHE["nc"] = _build()
    return _CACHE["nc"]


def kernel(x, attention_mask, Wq, Wk, Wv, Wr, Wff, bff):
    from concourse import bass_utils

    x = np.asarray(x, dtype=np.float32)
    attention_mask = np.asarray(attention_mask)
    Wq = np.asarray(Wq, dtype=np.float32)
    Wk = np.asarray(Wk, dtype=np.float32)
    Wv = np.asarray(Wv, dtype=np.float32)
    Wr = np.asarray(Wr, dtype=np.float32)
    Wff = np.asarray(Wff, dtype=np.float32)
    bff = np.asarray(bff, dtype=np.float32)

    if not np.all(attention_mask == 1):
        return _np_reference(x, attention_mask, Wq, Wk, Wv, Wr, Wff, bff)

    nc = _get_module()
    wq2 = np.ascontiguousarray(Wq.transpose(1, 0, 2).reshape(E, E))
    wk2 = np.ascontiguousarray(Wk.transpose(1, 0, 2).reshape(E, E))
    wv2 = np.ascontiguousarray(Wv.transpose(1, 0, 2).reshape(E, E))
    wfft = np.ascontiguousarray(Wff.T)
    bff2 = np.ascontiguousarray(bff.reshape(E, 1))

    in_maps = []
    for c in range(8):
        b, nh = c // 2, c % 2
        xt = x[b].T
        if nh:
            xt = np.concatenate([xt[:, NH:], xt[:, :NH]], axis=1)
        in_maps.append({
            "xt": np.ascontiguousarray(xt),
            "wq": wq2, "wk": wk2, "wv": wv2,
            "wr": np.ascontiguousarray(Wr[b]),
            "wfft": wfft, "bff": bff2,
            "cst": _CST,
        })

    res = bass_utils.run_bass_kernel_spmd(nc, in_maps, core_ids=list(range(8)), **_CACHE.get("run_kwargs", {}))
    _CACHE["last_result"] = res

    out = np.empty((B, S, E), dtype=np.float32)
    for c in range(8):
        b, nh = c // 2, c % 2
        out[b, nh * NH:(nh + 1) * NH, :] = res.results[c]["o"].T
    return out
